# revision 24
# baseline (speedup 1.0000x reference)
"""Trainium2 Bass kernel for BigSSMBlock (B=1, T=1024, d=1024, ds=64), 8-core SPMD.

T-sharded: each core owns a 128-timestep slice for the projections
and intra-chunk scan; ONE AllGather moves per-core scan-state summaries
(U_k, la_sum_k) so every core reconstructs its incoming state locally.
Weights are replicated (bf16); per-channel ops loop over 8 channel groups.
"""
import numpy as np

import concourse.bass as bass
import concourse.bacc as bacc
import concourse.mybir as mybir
import concourse.tile as tile
from concourse import bass_utils, masks

F32 = mybir.dt.float32
BF16 = mybir.dt.bfloat16

D = 1024
T = 1024
DS = 64
NCORE = 8
TS = T // NCORE          # 128 timesteps per core
HALO = 3
NA = D // 128            # 8 channel groups of 128
EPS = 1e-6
THR = -88.722839
CLAMP = -80.0

TRACE = False
LAST_RESULT = None
_CACHE = {}


def _build():
    if "nc" in _CACHE:
        return _CACHE["nc"]
    nc = bacc.Bacc("TRN2", target_bir_lowering=False, debug=False,
                   num_devices=NCORE)

    TW = 136  # TS + HALO, padded to 32B alignment
    xTs = nc.dram_tensor("xTs", [D, TW], BF16, kind="ExternalInput")
    xres = nc.dram_tensor("xres", [128, NA * TS], F32, kind="ExternalInput")
    w_inT = nc.dram_tensor("w_inT", [D, 2 * D], BF16, kind="ExternalInput")
    dt_wT = nc.dram_tensor("dt_wT", [D, D], BF16, kind="ExternalInput")
    bc_wT = nc.dram_tensor("bc_wT", [D, 2 * DS], BF16, kind="ExternalInput")
    out_wT = nc.dram_tensor("out_wT", [D, D], BF16, kind="ExternalInput")
    ppt = nc.dram_tensor("ppt", [128, NA * 8], F32, kind="ExternalInput")
    cmask = nc.dram_tensor("cmask", [128, 16], F32, kind="ExternalInput")
    triu = nc.dram_tensor("triu", [128, 128], F32, kind="ExternalInput")
    out = nc.dram_tensor("out", [128, NA * TS], F32, kind="ExternalOutput")

    AFT = mybir.ActivationFunctionType
    OP = mybir.AluOpType

    with tile.TileContext(nc) as tc:
        with (
            tc.tile_pool(name="const", bufs=1) as cpool,
            tc.tile_pool(name="wpool", bufs=1) as wpool,
            tc.tile_pool(name="act", bufs=1) as apool,
            tc.tile_pool(name="dram", bufs=1, space="DRAM") as dpool,
        ):
            # ---- inputs ----
            XT = wpool.tile([128, NA, TW], BF16)
            nc.sync.dma_start(XT[:], xTs.ap().rearrange("(a p) n -> p a n", p=128))
            W_in = wpool.tile([128, NA, 2 * D], BF16)
            nc.sync.dma_start(W_in[:], w_inT.ap().rearrange("(a p) n -> p a n", p=128))
            PPT = wpool.tile([128, NA, 8], F32)
            nc.gpsimd.dma_start(PPT[:], ppt.ap().rearrange("p (a n) -> p a n", a=NA))
            CMASK = wpool.tile([128, 16], F32)
            nc.gpsimd.dma_start(CMASK[:], cmask[:])
            TRIU = cpool.tile([128, 128], F32)
            nc.gpsimd.dma_start(TRIU[:], triu[:])
            W_dt = wpool.tile([128, NA, D], BF16)
            nc.sync.dma_start(W_dt[:], dt_wT.ap().rearrange("(a p) n -> p a n", p=128))
            W_bc = wpool.tile([128, NA, 2 * DS], BF16)
            nc.scalar.dma_start(W_bc[:], bc_wT.ap().rearrange("(a p) n -> p a n", p=128))
            W_out = wpool.tile([128, NA, D], BF16)
            nc.sync.dma_start(W_out[:], out_wT.ap().rearrange("(a p) n -> p a n", p=128))

            # ---- constants ----
            ident = cpool.tile([128, 128], F32)
            masks.make_identity(nc, ident[:])
            onesf = cpool.tile([1, 128], F32)
            nc.vector.memset(onesf[:], 1.0)
            ones_bf = cpool.tile([128, 1], BF16)
            nc.vector.memset(ones_bf[:], 1.0)
            nan_t = cpool.tile([128, NA * TS], F32)
            nc.vector.memset(nan_t[:], float("nan"))
            eps1 = cpool.tile([1, 1], F32)
            nc.vector.memset(eps1[:], EPS)
            wu_in = dpool.tile([128, 1], F32)
            wu_out = dpool.tile([NCORE * 128, 1], F32, addr_space="Shared")
            nc.gpsimd.dma_start(wu_in[:], ident[:, 0:1])
            nc.gpsimd.collective_compute(
                "AllGather", OP.bypass,
                replica_groups=[list(range(NCORE))],
                ins=[wu_in.opt()], outs=[wu_out.opt()])

            # ---- rms over d for the 131 local timesteps ----
            ps_rms = tc.tile_pool(name="ps_rms", bufs=1, space="PSUM")
            psA = ps_rms.__enter__()
            R = psA.tile([1, TW], F32)
            SQ = apool.tile([128, NA, TW], BF16, name="SQ")
            nc.scalar.activation(SQ[:], XT[:], AFT.Square)
            for k in range(NA):
                nc.tensor.matmul(R[:], ones_bf[:], SQ[:, k, :],
                                 start=(k == 0), stop=(k == NA - 1))
            lrow = apool.tile([1, TW], F32, name="lrow")
            nc.scalar.activation(lrow[:], R[:], AFT.Ln, bias=eps1[:], scale=1.0 / D)
            sinv = apool.tile([1, TW], F32, name="sinv")
            nc.scalar.activation(sinv[:], lrow[:], AFT.Exp, scale=-0.5)
            SBp = psA.tile([128, TW], F32, name="SBp")
            nc.tensor.matmul(SBp[:], onesf[:], sinv[:], start=True, stop=True)
            SB = apool.tile([128, TW], F32, name="SB")
            nc.vector.tensor_copy(SB[:], SBp[:])
            ps_rms.__exit__(None, None, None)

            # ---- in_proj (xs on 131 cols with halo; z on 128 cols) ----
            ps_xz = tc.tile_pool(name="ps_xz", bufs=8, space="PSUM")
            psB = ps_xz.__enter__()
            CONVIN = apool.tile([128, NA, TW], F32, name="CONVIN")
            ZSC = apool.tile([128, NA, TS], F32, name="ZSC")
            for j in range(NA):
                xp = psB.tile([128, TW], F32, name="xp", tag="xp", bufs=8)
                for k in range(NA):
                    nc.tensor.matmul(xp[:], W_in[:, k, j * 128:(j + 1) * 128],
                                     XT[:, k, :], start=(k == 0), stop=(k == NA - 1))
                nc.vector.tensor_mul(CONVIN[:, j, :], xp[:], SB[:])
            ps_xz.__exit__(None, None, None)

            # ---- conv + silu ----
            CV = apool.tile([128, NA, TS], F32, name="CV")
            for a in range(NA):
                nc.vector.tensor_scalar(CV[:, a, :], CONVIN[:, a, 0:TS],
                                        PPT[:, a, 4:5], PPT[:, a, 2:3],
                                        OP.mult, OP.add)
                for k in range(1, 4):
                    nc.vector.scalar_tensor_tensor(
                        CV[:, a, :], CONVIN[:, a, k:k + TS], PPT[:, a, 4 + k:5 + k],
                        CV[:, a, :], OP.mult, OP.add)
            XS2 = apool.tile([128, NA, TS], F32, name="XS2")
            nc.scalar.activation(XS2[:], CV[:], AFT.Silu)
            XS2B = apool.tile([128, NA, TS], BF16, name="XS2B")
            nc.vector.tensor_copy(XS2B[:], XS2[:])

            # ---- dt / BC projections (local full-d contraction) ----
            ps_dt = tc.tile_pool(name="ps_dt", bufs=8, space="PSUM")
            psD = ps_dt.__enter__()
            DTt = apool.tile([128, NA, TS], F32, name="DTt")
            for j in range(NA):
                dp = psD.tile([128, TS], F32, name="dp", tag="dp", bufs=8)
                for k in range(NA):
                    nc.tensor.matmul(dp[:], W_dt[:, k, j * 128:(j + 1) * 128],
                                     XS2B[:, k, :], start=(k == 0), stop=(k == NA - 1))
                # softplus = ln(1 + exp(pre + dt_b))
                nc.scalar.activation(DTt[:, j, :], dp[:], AFT.Exp,
                                     bias=PPT[:, j, 1:2])
            nc.scalar.activation(DTt[:], DTt[:], AFT.Ln, bias=1.0)
            bp = psD.tile([2 * DS, TS], F32, name="bp", tag="dp", bufs=8)
            for k in range(NA):
                nc.tensor.matmul(bp[:], W_bc[:, k, :], XS2B[:, k, :],
                                 start=(k == 0), stop=(k == NA - 1))
            BM = apool.tile([DS, TS], F32, name="BM")
            nc.vector.tensor_copy(BM[:], bp[0:DS, :])
            CM = apool.tile([DS, TS], F32, name="CM")
            nc.vector.tensor_copy(CM[:], bp[DS:2 * DS, :])
            ps_dt.__exit__(None, None, None)

            # ---- la, local cumsum, dtx ----
            DTX = apool.tile([128, NA, TS], F32, name="DTX")
            nc.vector.tensor_mul(DTX[:], DTt[:], XS2[:])
            LA = apool.tile([128, NA, TS], F32, name="LA")
            for a in range(NA):
                nc.vector.tensor_scalar(LA[:, a, :], DTt[:, a, :],
                                        PPT[:, a, 0:1], None, OP.mult)
            nc.vector.tensor_scalar(LA[:], LA[:], 20.0, -20.0, OP.min, OP.max)
            CUML = apool.tile([128, NA, TS], F32, name="CUML")
            for a in range(NA):
                nc.vector.tensor_tensor_scan(CUML[:, a, :], LA[:, a, :],
                                             LA[:, a, :], 0.0, OP.add, OP.bypass)
            LSUM = CUML  # [:, a, TS-1] slices

            # E = exp(-cl) (d,t); wT/clT/dtxT transposed per group
            ps_s = tc.tile_pool(name="ps_s", bufs=1, space="PSUM")
            psS = ps_s.__enter__()
            EE = apool.tile([128, NA, TS], F32, name="EE")
            nc.scalar.activation(EE[:], CUML[:], AFT.Exp, scale=-1.0)
            nc.vector.tensor_scalar(EE[:], EE[:], 5.5e34, None, OP.min)
            WV = apool.tile([128, NA, TS], F32, name="WV")
            nc.vector.tensor_mul(WV[:], EE[:], DTX[:])
            WVT = apool.tile([128, NA, 128], F32, name="WVT")
            PTT = apool.tile([128, NA, 128], F32, name="PTT")
            for a in range(NA):
                tw = psS.tile([128, 128], F32, name="tw", tag="tw", bufs=3)
                nc.tensor.transpose(tw[:], WV[:, a, :], ident[:])
                nc.vector.tensor_copy(WVT[:, a, :], tw[:])
                tc2 = psS.tile([128, 128], F32, name="tw", tag="tw", bufs=3)
                nc.tensor.transpose(tc2[:], CUML[:, a, :], ident[:])
                nc.scalar.activation(PTT[:, a, :], tc2[:], AFT.Exp)
            tb = psS.tile([128, DS], F32, name="tb", tag="tw", bufs=3)
            nc.tensor.transpose(tb[:, 0:DS], BM[:], ident[0:DS, 0:DS])
            BMT = apool.tile([128, DS], F32, name="BMT")
            nc.vector.tensor_copy(BMT[:], tb[:, 0:DS])

            # U_k[d, s] = sum_t wv[t, d] * Bm[s, t]; la_sum — pack and AG
            AGIN = apool.tile([128, NA * DS + NA], F32, name="AGIN")
            for a in range(NA):
                uu = psS.tile([128, DS], F32, name="uu", tag="tw", bufs=3)
                nc.tensor.matmul(uu[:], WVT[:, a, :], BMT[:], start=True, stop=True)
                nc.vector.tensor_copy(AGIN[:, a * DS:(a + 1) * DS], uu[:])
                nc.vector.tensor_copy(AGIN[:, NA * DS + a:NA * DS + a + 1],
                                      CUML[:, a, TS - 1:TS])
            ag_in = dpool.tile([128, NA * DS + NA], F32)
            ag_out = dpool.tile([NCORE * 128, NA * DS + NA], F32,
                                addr_space="Shared")
            nc.sync.dma_start(ag_in[:], AGIN[:])
            nc.gpsimd.collective_compute(
                "AllGather", OP.bypass,
                replica_groups=[list(range(NCORE))],
                ins=[ag_in.opt()], outs=[ag_out.opt()])

            # ---- overlap AG: G matrix + y1 (state-independent) ----
            gp = psS.tile([128, 128], F32, name="gp", tag="tw", bufs=3)
            nc.tensor.matmul(gp[:], BM[:], CM[:], start=True, stop=True)
            GM = apool.tile([128, 128], F32, name="GM")
            nc.vector.tensor_mul(GM[:], gp[:], TRIU[:])
            ps_z = tc.tile_pool(name="ps_z", bufs=1, space="PSUM")
            psZ = ps_z.__enter__()
            for j in range(NA):
                zp = psZ.tile([128, TS], F32, name="zp", tag="zp", bufs=1)
                for k in range(NA):
                    nc.tensor.matmul(
                        zp[:], W_in[:, k, D + j * 128:D + (j + 1) * 128],
                        XT[:, k, HALO:HALO + TS], start=(k == 0), stop=(k == NA - 1))
                nc.vector.tensor_mul(ZSC[:, j, :], zp[:], SB[:, HALO:HALO + TS])
            ps_z.__exit__(None, None, None)
            SZ = apool.tile([128, NA, TS], F32, name="SZ")
            nc.scalar.activation(SZ[:], ZSC[:], AFT.Silu)
            Y1 = apool.tile([128, NA, 128], F32, name="Y1")
            for a in range(NA):
                y1p = psS.tile([128, 128], F32, name="y1p", tag="tw", bufs=3)
                nc.tensor.matmul(y1p[:], GM[:], WVT[:, a, :], start=True, stop=True)
                nc.vector.tensor_copy(Y1[:, a, :], y1p[:])

            # ---- AG readback + masked prefix combine ----
            GU = wpool.tile([128, NCORE, NA * DS + NA], F32)
            nc.sync.dma_start(GU[:], ag_out[:].rearrange("(j p) n -> p j n", p=128))
            S = apool.tile([128, NA * DS], F32, name="S")
            nc.vector.memset(S[:], 0.0)
            OFF = apool.tile([128, NA], F32, name="OFF")
            nc.vector.memset(OFF[:], 0.0)
            AJ = apool.tile([128, NA], F32, name="AJ")
            AJm = apool.tile([128, NA], F32, name="AJm")
            for j in range(NCORE - 1):
                nc.scalar.activation(AJ[:], GU[:, j, NA * DS:NA * DS + NA], AFT.Exp)
                # A'_j = A_j * m + (1 - m)
                nc.vector.tensor_scalar(AJm[:], AJ[:], CMASK[:, j:j + 1],
                                        CMASK[:, 8 + j:9 + j], OP.mult, OP.add)
                # S = S * bcast(A'_j) + U_j * m
                S3 = S[:].rearrange("p (a s) -> p a s", a=NA)
                nc.vector.tensor_mul(S3, S3, AJm[:].to_broadcast((128, NA, DS)))
                nc.vector.scalar_tensor_tensor(
                    S3, GU[:, j, 0:NA * DS].rearrange("p (a s) -> p a s", a=NA),
                    CMASK[:, j:j + 1], S3, OP.mult, OP.add)
                # off += la_sum_j * m
                nc.vector.scalar_tensor_tensor(
                    OFF[:], GU[:, j, NA * DS:NA * DS + NA], CMASK[:, j:j + 1],
                    OFF[:], OP.mult, OP.add)

            # poison mask from global cum = CUML + off
            MASK = apool.tile([128, NA, TS], mybir.dt.uint8, name="MASK")
            for a in range(NA):
                nc.vector.tensor_scalar(MASK[:, a, :], CUML[:, a, :],
                                        OFF[:, a:a + 1], THR, OP.add, OP.is_lt)

            # ---- y2 = Cm @ S^T, scale, transpose back ----
            YS = apool.tile([128, NA, TS], F32, name="YS")
            for a in range(NA):
                stp = psS.tile([DS, 128], F32, name="stp", tag="tw", bufs=3)
                nc.tensor.transpose(stp[:], S[:, a * DS:(a + 1) * DS], ident[:])
                ST = apool.tile([DS, 128], F32, name="ST", bufs=2)
                nc.vector.tensor_copy(ST[:], stp[:])
                yp = psS.tile([128, 128], F32, name="yp", tag="yp", bufs=2)
                nc.tensor.matmul(yp[:], CM[:], ST[:], start=True, stop=True)
                YT = apool.tile([128, 128], F32, name="YT", bufs=2)
                nc.vector.tensor_add(YT[:], yp[:], Y1[:, a, :])
                nc.vector.tensor_mul(YT[:], YT[:], PTT[:, a, :])
                yb = psS.tile([128, 128], F32, name="yb", tag="yb", bufs=2)
                nc.tensor.transpose(yb[:], YT[:], ident[:])
                nc.vector.tensor_copy(YS[:, a, :], yb[:])
            ps_s.__exit__(None, None, None)

            # ---- gating + poison + out_proj ----
            YD = apool.tile([128, NA, TS], F32, name="YD")
            for a in range(NA):
                nc.vector.scalar_tensor_tensor(YD[:, a, :], XS2[:, a, :],
                                               PPT[:, a, 3:4], YS[:, a, :],
                                               OP.mult, OP.add)
            YF = apool.tile([128, NA, TS], F32, name="YF")
            nc.vector.tensor_mul(YF[:], YD[:], SZ[:])
            nc.vector.copy_predicated(
                YF[:].rearrange("p a t -> p (a t)"),
                MASK[:].rearrange("p a t -> p (a t)"), nan_t[:])
            YFB = apool.tile([128, NA, TS], BF16, name="YFB")
            nc.vector.tensor_copy(YFB[:], YF[:])

            XRES = wpool.tile([128, NA, TS], F32)
            nc.sync.dma_start(XRES[:], xres.ap().rearrange("p (a n) -> p a n", a=NA))
            ps_o = tc.tile_pool(name="ps_o", bufs=8, space="PSUM")
            psO = ps_o.__enter__()
            OUT = apool.tile([128, NA, TS], F32, name="OUT")
            for j in range(NA):
                op_ = psO.tile([128, TS], F32, name="op", tag="op", bufs=8)
                for k in range(NA):
                    nc.tensor.matmul(op_[:], W_out[:, k, j * 128:(j + 1) * 128],
                                     YFB[:, k, :], start=(k == 0), stop=(k == NA - 1))
                nc.vector.tensor_add(OUT[:, j, :], op_[:], XRES[:, j, :])
            ps_o.__exit__(None, None, None)
            nc.sync.dma_start(out.ap().rearrange("p (a n) -> p a n", a=NA), OUT[:])

    nc.compile()
    _CACHE["nc"] = nc
    return nc


def kernel(x, norm_w, in_proj_w, conv_w, conv_b, dt_w, dt_b, B_w, C_w, out_w,
           log_A, D: np.ndarray = None, **kw):
    import ml_dtypes
    global LAST_RESULT
    bf = ml_dtypes.bfloat16
    Dv = D if D is not None else kw["D"]
    f32 = np.float32

    nc = _build()
    xT = np.ascontiguousarray(np.asarray(x, f32)[0].T)          # (d, T)
    xTp = np.concatenate([np.zeros((1024, HALO), f32), xT], axis=1)
    nw = np.asarray(norm_w, f32)
    W_in_f = np.asarray(in_proj_w, f32) * nw[None, :]
    w_inT = np.ascontiguousarray(W_in_f.T).astype(bf)           # (d, 2d)
    dt_wTf = np.ascontiguousarray(np.asarray(dt_w, f32).T).astype(bf)
    out_wTf = np.ascontiguousarray(np.asarray(out_w, f32).T).astype(bf)
    bc_wT = np.ascontiguousarray(
        np.concatenate([np.asarray(B_w, f32).T, np.asarray(C_w, f32).T],
                       axis=1)).astype(bf)
    A = (-np.exp(np.asarray(log_A, f32))).astype(f32)
    cw = np.asarray(conv_w, f32)[:, 0, :]
    ppt = np.stack([A, np.asarray(dt_b, f32), np.asarray(conv_b, f32),
                    np.asarray(Dv, f32), cw[:, 0], cw[:, 1], cw[:, 2],
                    cw[:, 3]], axis=1).astype(np.float32)        # (d, 8)
    ppt = np.ascontiguousarray(
        ppt.reshape(NA, 128, 8).transpose(1, 0, 2).reshape(128, NA * 8))
    triu_m = np.triu(np.ones((128, 128), np.float32))

    in_maps = []
    for k in range(NCORE):
        t0 = k * TS
        xs_slice = np.zeros((1024, 136), np.float32)
        xs_slice[:, 0:TS + HALO] = xTp[:, t0:t0 + TS + HALO]
        xs_slice = xs_slice.astype(bf)
        cm = np.zeros((128, 16), np.float32)
        cm[:, 0:NCORE - 1] = (np.arange(NCORE - 1) < k).astype(np.float32)[None, :]
        cm[:, 8:8 + NCORE - 1] = 1.0 - cm[:, 0:NCORE - 1]
        in_maps.append(dict(
            xTs=xs_slice,
            xres=np.ascontiguousarray(
                xT[:, t0:t0 + TS].reshape(NA, 128, TS).transpose(1, 0, 2)
                .reshape(128, NA * TS)),
            w_inT=w_inT, dt_wT=dt_wTf, bc_wT=bc_wT, out_wT=out_wTf,
            ppt=ppt, cmask=cm, triu=triu_m))

    res = bass_utils.run_bass_kernel_spmd(nc, in_maps,
                                          core_ids=list(range(NCORE)),
                                          trace=TRACE)
    LAST_RESULT = res
    cols = []
    for k in range(NCORE):
        o = res.results[k]["out"].reshape(128, NA, TS).transpose(1, 0, 2)
        cols.append(o.reshape(1024, TS))
    full = np.concatenate(cols, axis=1)
    return np.ascontiguousarray(full.T)[None].astype(np.float32)


# revision 26
# speedup vs baseline: 1.1133x; 1.1133x over previous
"""Trainium2 Bass kernel for BigSSMBlock (B=1, T=1024, d=1024, ds=64), 8-core SPMD.

T-sharded: each core owns a 128-timestep slice for the projections
and intra-chunk scan; ONE AllGather moves per-core scan-state summaries
(U_k, la_sum_k) so every core reconstructs its incoming state locally.
Weights are replicated (bf16); per-channel ops loop over 8 channel groups.
"""
import numpy as np

import concourse.bass as bass
import concourse.bacc as bacc
import concourse.mybir as mybir
import concourse.tile as tile
from concourse import bass_utils, masks

F32 = mybir.dt.float32
BF16 = mybir.dt.bfloat16

D = 1024
T = 1024
DS = 64
NCORE = 8
TS = T // NCORE          # 128 timesteps per core
HALO = 3
NA = D // 128            # 8 channel groups of 128
EPS = 1e-6
THR = -88.722839
CLAMP = -80.0

TRACE = False
LAST_RESULT = None
_CACHE = {}


def _build():
    if "nc" in _CACHE:
        return _CACHE["nc"]
    nc = bacc.Bacc("TRN2", target_bir_lowering=False, debug=False,
                   num_devices=NCORE)

    TW = 136  # TS + HALO, padded to 32B alignment
    xTs = nc.dram_tensor("xTs", [D, TW], BF16, kind="ExternalInput")
    xres = nc.dram_tensor("xres", [128, NA * TS], F32, kind="ExternalInput")
    w_inT = nc.dram_tensor("w_inT", [D, 2 * D], BF16, kind="ExternalInput")
    dt_wT = nc.dram_tensor("dt_wT", [D, D], BF16, kind="ExternalInput")
    bc_wT = nc.dram_tensor("bc_wT", [D, 2 * DS], BF16, kind="ExternalInput")
    out_wT = nc.dram_tensor("out_wT", [D, D], BF16, kind="ExternalInput")
    ppt = nc.dram_tensor("ppt", [128, NA * 8], F32, kind="ExternalInput")
    cmask = nc.dram_tensor("cmask", [128, 16], F32, kind="ExternalInput")
    triu = nc.dram_tensor("triu", [128, 128], F32, kind="ExternalInput")
    out = nc.dram_tensor("out", [128, NA * TS], F32, kind="ExternalOutput")

    AFT = mybir.ActivationFunctionType
    OP = mybir.AluOpType

    with tile.TileContext(nc) as tc:
        with (
            tc.tile_pool(name="const", bufs=1) as cpool,
            tc.tile_pool(name="wpool", bufs=1) as wpool,
            tc.tile_pool(name="act", bufs=1) as apool,
            tc.tile_pool(name="dram", bufs=1, space="DRAM") as dpool,
        ):
            # ---- inputs ----
            XT = wpool.tile([128, NA, TW], BF16)
            nc.sync.dma_start(XT[:], xTs.ap().rearrange("(a p) n -> p a n", p=128))
            W_in = wpool.tile([128, NA, 2 * D], BF16)
            nc.sync.dma_start(W_in[:], w_inT.ap().rearrange("(a p) n -> p a n", p=128))
            PPT = wpool.tile([128, NA, 8], F32)
            nc.gpsimd.dma_start(PPT[:], ppt.ap().rearrange("p (a n) -> p a n", a=NA))
            CMASK = wpool.tile([128, 16], F32)
            nc.gpsimd.dma_start(CMASK[:], cmask[:])
            TRIU = cpool.tile([128, 128], F32)
            nc.gpsimd.dma_start(TRIU[:], triu[:])
            W_dt = wpool.tile([128, NA, D], BF16)
            nc.sync.dma_start(W_dt[:], dt_wT.ap().rearrange("(a p) n -> p a n", p=128))
            W_bc = wpool.tile([128, NA, 2 * DS], BF16)
            nc.scalar.dma_start(W_bc[:], bc_wT.ap().rearrange("(a p) n -> p a n", p=128))
            W_out = wpool.tile([128, NA, D], BF16)
            nc.sync.dma_start(W_out[:], out_wT.ap().rearrange("(a p) n -> p a n", p=128))

            # ---- constants ----
            ident = cpool.tile([128, 128], F32)
            masks.make_identity(nc, ident[:])
            onesf = cpool.tile([1, 128], F32)
            nc.vector.memset(onesf[:], 1.0)
            ones_bf = cpool.tile([128, 1], BF16)
            nc.vector.memset(ones_bf[:], 1.0)
            nan_t = cpool.tile([128, NA * TS], F32)
            nc.vector.memset(nan_t[:], float("nan"))
            eps1 = cpool.tile([1, 1], F32)
            nc.vector.memset(eps1[:], EPS)
            wu_in = dpool.tile([128, 1], F32)
            wu_out = dpool.tile([NCORE * 128, 1], F32, addr_space="Shared")
            nc.gpsimd.dma_start(wu_in[:], ident[:, 0:1])
            nc.gpsimd.collective_compute(
                "AllGather", OP.bypass,
                replica_groups=[list(range(NCORE))],
                ins=[wu_in.opt()], outs=[wu_out.opt()])

            # ---- rms over d for the 131 local timesteps ----
            ps_rms = tc.tile_pool(name="ps_rms", bufs=1, space="PSUM")
            psA = ps_rms.__enter__()
            R = psA.tile([1, TW], F32)
            SQ = apool.tile([128, NA, TW], BF16, name="SQ")
            nc.scalar.activation(SQ[:], XT[:], AFT.Square)
            for k in range(NA):
                nc.tensor.matmul(R[:], ones_bf[:], SQ[:, k, :],
                                 start=(k == 0), stop=(k == NA - 1))
            lrow = apool.tile([1, TW], F32, name="lrow")
            nc.scalar.activation(lrow[:], R[:], AFT.Ln, bias=eps1[:], scale=1.0 / D)
            sinv = apool.tile([1, TW], F32, name="sinv")
            nc.scalar.activation(sinv[:], lrow[:], AFT.Exp, scale=-0.5)
            SBp = psA.tile([128, TW], F32, name="SBp")
            nc.tensor.matmul(SBp[:], onesf[:], sinv[:], start=True, stop=True)
            SB = apool.tile([128, TW], F32, name="SB")
            nc.vector.tensor_copy(SB[:], SBp[:])
            ps_rms.__exit__(None, None, None)

            # ---- in_proj (xs on 131 cols with halo; z on 128 cols) ----
            ps_xz = tc.tile_pool(name="ps_xz", bufs=8, space="PSUM")
            psB = ps_xz.__enter__()
            CONVIN = apool.tile([128, NA, TW], F32, name="CONVIN")
            ZSC = apool.tile([128, NA, TS], F32, name="ZSC")
            for j in range(NA):
                xp = psB.tile([128, TW], F32, name="xp", tag="xp", bufs=8)
                for k in range(NA):
                    nc.tensor.matmul(xp[:], W_in[:, k, j * 128:(j + 1) * 128],
                                     XT[:, k, :], start=(k == 0), stop=(k == NA - 1))
                nc.vector.tensor_mul(CONVIN[:, j, :], xp[:], SB[:])
            ps_xz.__exit__(None, None, None)

            # ---- conv + silu ----
            CV = apool.tile([128, NA, TS], F32, name="CV")
            for a in range(NA):
                nc.vector.tensor_scalar(CV[:, a, :], CONVIN[:, a, 0:TS],
                                        PPT[:, a, 4:5], PPT[:, a, 2:3],
                                        OP.mult, OP.add)
                for k in range(1, 4):
                    nc.vector.scalar_tensor_tensor(
                        CV[:, a, :], CONVIN[:, a, k:k + TS], PPT[:, a, 4 + k:5 + k],
                        CV[:, a, :], OP.mult, OP.add)
            XS2 = apool.tile([128, NA, TS], F32, name="XS2")
            nc.scalar.activation(XS2[:], CV[:], AFT.Silu)
            XS2B = apool.tile([128, NA, TS], BF16, name="XS2B")
            nc.vector.tensor_copy(XS2B[:], XS2[:])

            # ---- dt / BC projections (local full-d contraction) ----
            ps_dt = tc.tile_pool(name="ps_dt", bufs=8, space="PSUM")
            psD = ps_dt.__enter__()
            DTt = apool.tile([128, NA, TS], F32, name="DTt")
            for j in range(NA):
                dp = psD.tile([128, TS], F32, name="dp", tag="dp", bufs=8)
                for k in range(NA):
                    nc.tensor.matmul(dp[:], W_dt[:, k, j * 128:(j + 1) * 128],
                                     XS2B[:, k, :], start=(k == 0), stop=(k == NA - 1))
                # softplus = ln(1 + exp(pre + dt_b))
                nc.scalar.activation(DTt[:, j, :], dp[:], AFT.Exp,
                                     bias=PPT[:, j, 1:2])
            nc.scalar.activation(DTt[:], DTt[:], AFT.Ln, bias=1.0)
            bp = psD.tile([2 * DS, TS], F32, name="bp", tag="dp", bufs=8)
            for k in range(NA):
                nc.tensor.matmul(bp[:], W_bc[:, k, :], XS2B[:, k, :],
                                 start=(k == 0), stop=(k == NA - 1))
            BM = apool.tile([DS, TS], F32, name="BM")
            nc.vector.tensor_copy(BM[:], bp[0:DS, :])
            CM = apool.tile([DS, TS], F32, name="CM")
            nc.vector.tensor_copy(CM[:], bp[DS:2 * DS, :])
            ps_dt.__exit__(None, None, None)

            # ---- la, local cumsum, dtx ----
            DTX = apool.tile([128, NA, TS], F32, name="DTX")
            nc.vector.tensor_mul(DTX[:], DTt[:], XS2[:])
            LA = apool.tile([128, NA, TS], F32, name="LA")
            for a in range(NA):
                nc.vector.tensor_scalar(LA[:, a, :], DTt[:, a, :],
                                        PPT[:, a, 0:1], None, OP.mult)
            nc.vector.tensor_scalar(LA[:], LA[:], 20.0, -20.0, OP.min, OP.max)
            CUML = apool.tile([128, NA, TS], F32, name="CUML")
            for a in range(NA):
                nc.vector.tensor_tensor_scan(CUML[:, a, :], LA[:, a, :],
                                             LA[:, a, :], 0.0, OP.add, OP.bypass)
            LSUM = CUML  # [:, a, TS-1] slices

            # E = exp(-cl) (d,t); wT/clT/dtxT transposed per group
            ps_s = tc.tile_pool(name="ps_s", bufs=1, space="PSUM")
            psS = ps_s.__enter__()
            EE = apool.tile([128, NA, TS], F32, name="EE")
            nc.scalar.activation(EE[:], CUML[:], AFT.Exp, scale=-1.0)
            nc.vector.tensor_scalar(EE[:], EE[:], 5.5e34, None, OP.min)
            WV = apool.tile([128, NA, TS], F32, name="WV")
            nc.vector.tensor_mul(WV[:], EE[:], DTX[:])
            WVT = apool.tile([128, NA, 128], F32, name="WVT")
            PTT = apool.tile([128, NA, 128], F32, name="PTT")
            for a in range(NA):
                tw = psS.tile([128, 128], F32, name="tw", tag="tw", bufs=3)
                nc.tensor.transpose(tw[:], WV[:, a, :], ident[:])
                nc.vector.tensor_copy(WVT[:, a, :], tw[:])
                tc2 = psS.tile([128, 128], F32, name="tw", tag="tw", bufs=3)
                nc.tensor.transpose(tc2[:], CUML[:, a, :], ident[:])
                nc.scalar.activation(PTT[:, a, :], tc2[:], AFT.Exp)
            tb = psS.tile([128, DS], F32, name="tb", tag="tw", bufs=3)
            nc.tensor.transpose(tb[:, 0:DS], BM[:], ident[0:DS, 0:DS])
            BMT = apool.tile([128, DS], F32, name="BMT")
            nc.vector.tensor_copy(BMT[:], tb[:, 0:DS])

            # U_k[d, s] = sum_t wv[t, d] * Bm[s, t]; la_sum — pack and AG
            AGIN = apool.tile([128, NA * DS + NA], F32, name="AGIN")
            for a in range(NA):
                uu = psS.tile([128, DS], F32, name="uu", tag="tw", bufs=3)
                nc.tensor.matmul(uu[:], WVT[:, a, :], BMT[:], start=True, stop=True)
                nc.vector.tensor_copy(AGIN[:, a * DS:(a + 1) * DS], uu[:])
                nc.vector.tensor_copy(AGIN[:, NA * DS + a:NA * DS + a + 1],
                                      CUML[:, a, TS - 1:TS])
            ag_in = dpool.tile([128, NA * DS + NA], F32)
            ag_out = dpool.tile([NCORE * 128, NA * DS + NA], F32,
                                addr_space="Shared")
            nc.sync.dma_start(ag_in[:], AGIN[:])
            nc.gpsimd.collective_compute(
                "AllGather", OP.bypass,
                replica_groups=[list(range(NCORE))],
                ins=[ag_in.opt()], outs=[ag_out.opt()])

            # ---- overlap AG: G matrix + y1 (state-independent) ----
            gp = psS.tile([128, 128], F32, name="gp", tag="tw", bufs=3)
            nc.tensor.matmul(gp[:], BM[:], CM[:], start=True, stop=True)
            GM = apool.tile([128, 128], F32, name="GM")
            nc.vector.tensor_mul(GM[:], gp[:], TRIU[:])
            ps_z = tc.tile_pool(name="ps_z", bufs=3, space="PSUM")
            psZ = ps_z.__enter__()
            for j in range(NA):
                zp = psZ.tile([128, TS], F32, name="zp", tag="zp", bufs=3)
                for k in range(NA):
                    nc.tensor.matmul(
                        zp[:], W_in[:, k, D + j * 128:D + (j + 1) * 128],
                        XT[:, k, HALO:HALO + TS], start=(k == 0), stop=(k == NA - 1))
                nc.vector.tensor_mul(ZSC[:, j, :], zp[:], SB[:, HALO:HALO + TS])
            ps_z.__exit__(None, None, None)
            SZ = apool.tile([128, NA, TS], F32, name="SZ")
            nc.scalar.activation(SZ[:], ZSC[:], AFT.Silu)
            Y1 = apool.tile([128, NA, 128], F32, name="Y1")
            for a in range(NA):
                y1p = psS.tile([128, 128], F32, name="y1p", tag="tw", bufs=3)
                nc.tensor.matmul(y1p[:], GM[:], WVT[:, a, :], start=True, stop=True)
                nc.vector.tensor_copy(Y1[:, a, :], y1p[:])

            # ---- AG readback + masked prefix combine ----
            GU = wpool.tile([128, NCORE, NA * DS + NA], F32)
            nc.sync.dma_start(GU[:], ag_out[:].rearrange("(j p) n -> p j n", p=128))
            S = apool.tile([128, NA * DS], F32, name="S")
            nc.vector.memset(S[:], 0.0)
            OFF = apool.tile([128, NA], F32, name="OFF")
            nc.vector.memset(OFF[:], 0.0)
            AJ = apool.tile([128, NA], F32, name="AJ")
            AJm = apool.tile([128, NA], F32, name="AJm")
            for j in range(NCORE - 1):
                nc.scalar.activation(AJ[:], GU[:, j, NA * DS:NA * DS + NA], AFT.Exp)
                # A'_j = A_j * m + (1 - m)
                nc.vector.tensor_scalar(AJm[:], AJ[:], CMASK[:, j:j + 1],
                                        CMASK[:, 8 + j:9 + j], OP.mult, OP.add)
                # S = S * bcast(A'_j) + U_j * m
                S3 = S[:].rearrange("p (a s) -> p a s", a=NA)
                nc.vector.tensor_mul(S3, S3, AJm[:].to_broadcast((128, NA, DS)))
                nc.vector.scalar_tensor_tensor(
                    S3, GU[:, j, 0:NA * DS].rearrange("p (a s) -> p a s", a=NA),
                    CMASK[:, j:j + 1], S3, OP.mult, OP.add)
                # off += la_sum_j * m
                nc.vector.scalar_tensor_tensor(
                    OFF[:], GU[:, j, NA * DS:NA * DS + NA], CMASK[:, j:j + 1],
                    OFF[:], OP.mult, OP.add)

            # poison mask from global cum = CUML + off
            MASK = apool.tile([128, NA, TS], mybir.dt.uint8, name="MASK")
            for a in range(NA):
                nc.vector.tensor_scalar(MASK[:, a, :], CUML[:, a, :],
                                        OFF[:, a:a + 1], THR, OP.add, OP.is_lt)

            # ---- y2 = Cm @ S^T, scale, transpose back ----
            YS = apool.tile([128, NA, TS], F32, name="YS")
            ST_all = apool.tile([DS, NA, 128], F32, name="ST_all")
            for a in range(NA):
                stp = psS.tile([DS, 128], F32, name="stp", tag="tw", bufs=3)
                nc.tensor.transpose(stp[:], S[:, a * DS:(a + 1) * DS], ident[:])
                nc.vector.tensor_copy(ST_all[:, a, :], stp[:])
            for a in range(NA):
                yp = psS.tile([128, 128], F32, name="yp", tag="yp", bufs=2)
                nc.tensor.matmul(yp[:], CM[:], ST_all[:, a, :],
                                 start=True, stop=True)
                YT = apool.tile([128, 128], F32, name="YT", bufs=2)
                nc.vector.tensor_add(YT[:], yp[:], Y1[:, a, :])
                nc.vector.tensor_mul(YT[:], YT[:], PTT[:, a, :])
                yb = psS.tile([128, 128], F32, name="yb", tag="tw", bufs=3)
                nc.tensor.transpose(yb[:], YT[:], ident[:])
                nc.vector.tensor_copy(YS[:, a, :], yb[:])
            ps_s.__exit__(None, None, None)

            # ---- gating + poison + out_proj ----
            YD = apool.tile([128, NA, TS], F32, name="YD")
            for a in range(NA):
                nc.vector.scalar_tensor_tensor(YD[:, a, :], XS2[:, a, :],
                                               PPT[:, a, 3:4], YS[:, a, :],
                                               OP.mult, OP.add)
            YF = apool.tile([128, NA, TS], F32, name="YF")
            nc.vector.tensor_mul(YF[:], YD[:], SZ[:])
            nc.vector.copy_predicated(
                YF[:].rearrange("p a t -> p (a t)"),
                MASK[:].rearrange("p a t -> p (a t)"), nan_t[:])
            YFB = apool.tile([128, NA, TS], BF16, name="YFB")
            nc.vector.tensor_copy(YFB[:], YF[:])

            XRES = wpool.tile([128, NA, TS], F32)
            nc.sync.dma_start(XRES[:], xres.ap().rearrange("p (a n) -> p a n", a=NA))
            ps_o = tc.tile_pool(name="ps_o", bufs=8, space="PSUM")
            psO = ps_o.__enter__()
            OUT = apool.tile([128, NA, TS], F32, name="OUT")
            for j in range(NA):
                op_ = psO.tile([128, TS], F32, name="op", tag="op", bufs=8)
                for k in range(NA):
                    nc.tensor.matmul(op_[:], W_out[:, k, j * 128:(j + 1) * 128],
                                     YFB[:, k, :], start=(k == 0), stop=(k == NA - 1))
                nc.vector.tensor_add(OUT[:, j, :], op_[:], XRES[:, j, :])
            ps_o.__exit__(None, None, None)
            nc.sync.dma_start(out.ap().rearrange("p (a n) -> p a n", a=NA), OUT[:])

    nc.compile()
    _CACHE["nc"] = nc
    return nc


def kernel(x, norm_w, in_proj_w, conv_w, conv_b, dt_w, dt_b, B_w, C_w, out_w,
           log_A, D: np.ndarray = None, **kw):
    import ml_dtypes
    global LAST_RESULT
    bf = ml_dtypes.bfloat16
    Dv = D if D is not None else kw["D"]
    f32 = np.float32

    nc = _build()
    xT = np.ascontiguousarray(np.asarray(x, f32)[0].T)          # (d, T)
    xTp = np.concatenate([np.zeros((1024, HALO), f32), xT], axis=1)
    nw = np.asarray(norm_w, f32)
    W_in_f = np.asarray(in_proj_w, f32) * nw[None, :]
    w_inT = np.ascontiguousarray(W_in_f.T).astype(bf)           # (d, 2d)
    dt_wTf = np.ascontiguousarray(np.asarray(dt_w, f32).T).astype(bf)
    out_wTf = np.ascontiguousarray(np.asarray(out_w, f32).T).astype(bf)
    bc_wT = np.ascontiguousarray(
        np.concatenate([np.asarray(B_w, f32).T, np.asarray(C_w, f32).T],
                       axis=1)).astype(bf)
    A = (-np.exp(np.asarray(log_A, f32))).astype(f32)
    cw = np.asarray(conv_w, f32)[:, 0, :]
    ppt = np.stack([A, np.asarray(dt_b, f32), np.asarray(conv_b, f32),
                    np.asarray(Dv, f32), cw[:, 0], cw[:, 1], cw[:, 2],
                    cw[:, 3]], axis=1).astype(np.float32)        # (d, 8)
    ppt = np.ascontiguousarray(
        ppt.reshape(NA, 128, 8).transpose(1, 0, 2).reshape(128, NA * 8))
    triu_m = np.triu(np.ones((128, 128), np.float32))

    in_maps = []
    for k in range(NCORE):
        t0 = k * TS
        xs_slice = np.zeros((1024, 136), np.float32)
        xs_slice[:, 0:TS + HALO] = xTp[:, t0:t0 + TS + HALO]
        xs_slice = xs_slice.astype(bf)
        cm = np.zeros((128, 16), np.float32)
        cm[:, 0:NCORE - 1] = (np.arange(NCORE - 1) < k).astype(np.float32)[None, :]
        cm[:, 8:8 + NCORE - 1] = 1.0 - cm[:, 0:NCORE - 1]
        in_maps.append(dict(
            xTs=xs_slice,
            xres=np.ascontiguousarray(
                xT[:, t0:t0 + TS].reshape(NA, 128, TS).transpose(1, 0, 2)
                .reshape(128, NA * TS)),
            w_inT=w_inT, dt_wT=dt_wTf, bc_wT=bc_wT, out_wT=out_wTf,
            ppt=ppt, cmask=cm, triu=triu_m))

    res = bass_utils.run_bass_kernel_spmd(nc, in_maps,
                                          core_ids=list(range(NCORE)),
                                          trace=TRACE)
    LAST_RESULT = res
    cols = []
    for k in range(NCORE):
        o = res.results[k]["out"].reshape(128, NA, TS).transpose(1, 0, 2)
        cols.append(o.reshape(1024, TS))
    full = np.concatenate(cols, axis=1)
    return np.ascontiguousarray(full.T)[None].astype(np.float32)


# revision 31
# speedup vs baseline: 1.1207x; 1.0066x over previous
"""Trainium2 Bass kernel for BigSSMBlock (B=1, T=1024, d=1024, ds=64), 8-core SPMD.

T-sharded: each core owns a 128-timestep slice for the projections
and intra-chunk scan; ONE AllGather moves per-core scan-state summaries
(U_k, la_sum_k) so every core reconstructs its incoming state locally.
Weights are replicated (bf16); per-channel ops loop over 8 channel groups.
"""
import numpy as np

import concourse.bass as bass
import concourse.bacc as bacc
import concourse.mybir as mybir
import concourse.tile as tile
from concourse import bass_utils, masks

F32 = mybir.dt.float32
BF16 = mybir.dt.bfloat16

D = 1024
T = 1024
DS = 64
NCORE = 8
TS = T // NCORE          # 128 timesteps per core
HALO = 3
NA = D // 128            # 8 channel groups of 128
EPS = 1e-6
THR = -88.722839
CLAMP = -80.0

TRACE = False
LAST_RESULT = None
_CACHE = {}


def _build():
    if "nc" in _CACHE:
        return _CACHE["nc"]
    nc = bacc.Bacc("TRN2", target_bir_lowering=False, debug=False,
                   num_devices=NCORE)

    TW = 136  # TS + HALO, padded to 32B alignment
    xTs = nc.dram_tensor("xTs", [D, TW], BF16, kind="ExternalInput")
    xres = nc.dram_tensor("xres", [128, NA * TS], F32, kind="ExternalInput")
    w_inT = nc.dram_tensor("w_inT", [D, 2 * D], BF16, kind="ExternalInput")
    dt_wT = nc.dram_tensor("dt_wT", [D, D], BF16, kind="ExternalInput")
    bc_wT = nc.dram_tensor("bc_wT", [D, 2 * DS], BF16, kind="ExternalInput")
    out_wT = nc.dram_tensor("out_wT", [D, D], BF16, kind="ExternalInput")
    ppt = nc.dram_tensor("ppt", [128, NA * 8], F32, kind="ExternalInput")
    cmask = nc.dram_tensor("cmask", [128, 16], F32, kind="ExternalInput")
    triu = nc.dram_tensor("triu", [128, 128], F32, kind="ExternalInput")
    out = nc.dram_tensor("out", [128, NA * TS], F32, kind="ExternalOutput")

    AFT = mybir.ActivationFunctionType
    OP = mybir.AluOpType

    with tile.TileContext(nc) as tc:
        with (
            tc.tile_pool(name="const", bufs=1) as cpool,
            tc.tile_pool(name="wpool", bufs=1) as wpool,
            tc.tile_pool(name="act", bufs=1) as apool,
            tc.tile_pool(name="dram", bufs=1, space="DRAM") as dpool,
        ):
            # ---- inputs ----
            XT = wpool.tile([128, NA, TW], BF16)
            nc.sync.dma_start(XT[:], xTs.ap().rearrange("(a p) n -> p a n", p=128))
            W_in = wpool.tile([128, NA, 2 * D], BF16)
            nc.sync.dma_start(W_in[:], w_inT.ap().rearrange("(a p) n -> p a n", p=128))
            PPT = wpool.tile([128, NA, 8], F32)
            nc.gpsimd.dma_start(PPT[:], ppt.ap().rearrange("p (a n) -> p a n", a=NA))
            CMASK = wpool.tile([128, 16], F32)
            nc.gpsimd.dma_start(CMASK[:], cmask[:])
            TRIU = cpool.tile([128, 128], F32)
            nc.gpsimd.dma_start(TRIU[:], triu[:])
            W_dt = wpool.tile([128, NA, D], BF16)
            nc.sync.dma_start(W_dt[:], dt_wT.ap().rearrange("(a p) n -> p a n", p=128))
            W_bc = wpool.tile([128, NA, 2 * DS], BF16)
            nc.scalar.dma_start(W_bc[:], bc_wT.ap().rearrange("(a p) n -> p a n", p=128))
            W_out = wpool.tile([128, NA, D], BF16)
            nc.sync.dma_start(W_out[:], out_wT.ap().rearrange("(a p) n -> p a n", p=128))

            # ---- constants ----
            ident = cpool.tile([128, 128], F32)
            masks.make_identity(nc, ident[:])
            onesf = cpool.tile([1, 128], F32)
            nc.vector.memset(onesf[:], 1.0)
            ones_bf = cpool.tile([128, 1], BF16)
            nc.vector.memset(ones_bf[:], 1.0)
            nan_t = cpool.tile([128, NA * TS], F32)
            nc.vector.memset(nan_t[:], float("nan"))
            eps1 = cpool.tile([1, 1], F32)
            nc.vector.memset(eps1[:], EPS)
            wu_in = dpool.tile([128, 1], F32)
            wu_out = dpool.tile([NCORE * 128, 1], F32, addr_space="Shared")
            nc.gpsimd.dma_start(wu_in[:], ident[:, 0:1])
            nc.gpsimd.collective_compute(
                "AllGather", OP.bypass,
                replica_groups=[list(range(NCORE))],
                ins=[wu_in.opt()], outs=[wu_out.opt()])

            # ---- rms over d for the 131 local timesteps ----
            ps_rms = tc.tile_pool(name="ps_rms", bufs=1, space="PSUM")
            psA = ps_rms.__enter__()
            R = psA.tile([1, TW], F32)
            SQ = apool.tile([128, NA, TW], BF16, name="SQ")
            nc.scalar.activation(SQ[:], XT[:], AFT.Square)
            for k in range(NA):
                nc.tensor.matmul(R[:], ones_bf[:], SQ[:, k, :],
                                 start=(k == 0), stop=(k == NA - 1))
            lrow = apool.tile([1, TW], F32, name="lrow")
            nc.scalar.activation(lrow[:], R[:], AFT.Ln, bias=eps1[:], scale=1.0 / D)
            sinv = apool.tile([1, TW], F32, name="sinv")
            nc.scalar.activation(sinv[:], lrow[:], AFT.Exp, scale=-0.5)
            SBp = psA.tile([128, TW], F32, name="SBp")
            nc.tensor.matmul(SBp[:], onesf[:], sinv[:], start=True, stop=True)
            SB = apool.tile([128, TW], F32, name="SB")
            nc.vector.tensor_copy(SB[:], SBp[:])
            ps_rms.__exit__(None, None, None)

            # ---- in_proj (xs on 131 cols with halo; z on 128 cols) ----
            ps_xz = tc.tile_pool(name="ps_xz", bufs=8, space="PSUM")
            psB = ps_xz.__enter__()
            CONVIN = apool.tile([128, NA, TW], F32, name="CONVIN")
            ZSC = apool.tile([128, NA, TS], F32, name="ZSC")
            for j in range(NA):
                xp = psB.tile([128, TW], F32, name="xp", tag="xp", bufs=8)
                for k in range(NA):
                    nc.tensor.matmul(xp[:], W_in[:, k, j * 128:(j + 1) * 128],
                                     XT[:, k, :], start=(k == 0), stop=(k == NA - 1))
                nc.vector.tensor_mul(CONVIN[:, j, :], xp[:], SB[:])
            ps_xz.__exit__(None, None, None)

            # ---- conv + silu ----
            CV = apool.tile([128, NA, TS], F32, name="CV")
            for a in range(NA):
                nc.vector.tensor_scalar(CV[:, a, :], CONVIN[:, a, 0:TS],
                                        PPT[:, a, 4:5], PPT[:, a, 2:3],
                                        OP.mult, OP.add)
                for k in range(1, 4):
                    nc.vector.scalar_tensor_tensor(
                        CV[:, a, :], CONVIN[:, a, k:k + TS], PPT[:, a, 4 + k:5 + k],
                        CV[:, a, :], OP.mult, OP.add)
            XS2 = apool.tile([128, NA, TS], F32, name="XS2")
            nc.scalar.activation(XS2[:], CV[:], AFT.Silu)
            XS2B = apool.tile([128, NA, TS], BF16, name="XS2B")
            nc.vector.tensor_copy(XS2B[:], XS2[:])

            # ---- dt / BC projections (local full-d contraction) ----
            ps_dt = tc.tile_pool(name="ps_dt", bufs=8, space="PSUM")
            psD = ps_dt.__enter__()
            DTt = apool.tile([128, NA, TS], F32, name="DTt")
            for j in range(NA):
                dp = psD.tile([128, TS], F32, name="dp", tag="dp", bufs=8)
                for k in range(NA):
                    nc.tensor.matmul(dp[:], W_dt[:, k, j * 128:(j + 1) * 128],
                                     XS2B[:, k, :], start=(k == 0), stop=(k == NA - 1))
                # softplus = ln(1 + exp(pre + dt_b))
                nc.scalar.activation(DTt[:, j, :], dp[:], AFT.Exp,
                                     bias=PPT[:, j, 1:2])
            nc.scalar.activation(DTt[:], DTt[:], AFT.Ln, bias=1.0)
            bp = psD.tile([2 * DS, TS], F32, name="bp", tag="dp", bufs=8)
            for k in range(NA):
                nc.tensor.matmul(bp[:], W_bc[:, k, :], XS2B[:, k, :],
                                 start=(k == 0), stop=(k == NA - 1))
            BM = apool.tile([DS, TS], F32, name="BM")
            nc.vector.tensor_copy(BM[:], bp[0:DS, :])
            CM = apool.tile([DS, TS], F32, name="CM")
            nc.vector.tensor_copy(CM[:], bp[DS:2 * DS, :])
            ps_dt.__exit__(None, None, None)

            # ---- la, local cumsum, dtx ----
            DTX = apool.tile([128, NA, TS], F32, name="DTX")
            nc.vector.tensor_mul(DTX[:], DTt[:], XS2[:])
            LA = apool.tile([128, NA, TS], F32, name="LA")
            for a in range(NA):
                nc.vector.tensor_scalar(LA[:, a, :], DTt[:, a, :],
                                        PPT[:, a, 0:1], None, OP.mult)
            nc.vector.tensor_scalar(LA[:], LA[:], 20.0, -20.0, OP.min, OP.max)
            CUML = apool.tile([128, NA, TS], F32, name="CUML")
            for a in range(NA):
                nc.vector.tensor_tensor_scan(CUML[:, a, :], LA[:, a, :],
                                             LA[:, a, :], 0.0, OP.add, OP.bypass)
            LSUM = CUML  # [:, a, TS-1] slices

            # E = exp(-cl) (d,t); wT/clT/dtxT transposed per group
            ps_s = tc.tile_pool(name="ps_s", bufs=1, space="PSUM")
            psS = ps_s.__enter__()
            EE = apool.tile([128, NA, TS], F32, name="EE")
            nc.scalar.activation(EE[:], CUML[:], AFT.Exp, scale=-1.0)
            nc.vector.tensor_scalar(EE[:], EE[:], 5.5e34, None, OP.min)
            WV = apool.tile([128, NA, TS], F32, name="WV")
            nc.vector.tensor_mul(WV[:], EE[:], DTX[:])
            WVT = apool.tile([128, NA, 128], F32, name="WVT")
            for a in range(NA):
                tw = psS.tile([128, 128], F32, name="tw", tag="tw", bufs=3)
                nc.tensor.transpose(tw[:], WV[:, a, :], ident[:])
                nc.vector.tensor_copy(WVT[:, a, :], tw[:])
            tb = psS.tile([128, DS], F32, name="tb", tag="tw", bufs=3)
            nc.tensor.transpose(tb[:, 0:DS], BM[:], ident[0:DS, 0:DS])
            BMT = apool.tile([128, DS], F32, name="BMT")
            nc.vector.tensor_copy(BMT[:], tb[:, 0:DS])

            # U_k[d, s] = sum_t wv[t, d] * Bm[s, t]; la_sum — pack and AG
            AGIN = apool.tile([128, NA * DS + NA], F32, name="AGIN")
            for a in range(NA):
                uu = psS.tile([128, DS], F32, name="uu", tag="tw", bufs=3)
                nc.tensor.matmul(uu[:], WVT[:, a, :], BMT[:], start=True, stop=True)
                nc.vector.tensor_copy(AGIN[:, a * DS:(a + 1) * DS], uu[:])
                nc.vector.tensor_copy(AGIN[:, NA * DS + a:NA * DS + a + 1],
                                      CUML[:, a, TS - 1:TS])
            ag_in = dpool.tile([128, NA * DS + NA], F32)
            ag_out = dpool.tile([NCORE * 128, NA * DS + NA], F32,
                                addr_space="Shared")
            nc.sync.dma_start(ag_in[:], AGIN[:])
            nc.gpsimd.collective_compute(
                "AllGather", OP.bypass,
                replica_groups=[list(range(NCORE))],
                ins=[ag_in.opt()], outs=[ag_out.opt()])
            PTT = apool.tile([128, NA, 128], F32, name="PTT")
            for a in range(NA):
                tc2 = psS.tile([128, 128], F32, name="tw", tag="tw", bufs=3)
                nc.tensor.transpose(tc2[:], CUML[:, a, :], ident[:])
                nc.scalar.activation(PTT[:, a, :], tc2[:], AFT.Exp)

            # ---- overlap AG: G matrix + y1 (state-independent) ----
            gp = psS.tile([128, 128], F32, name="gp", tag="tw", bufs=3)
            nc.tensor.matmul(gp[:], BM[:], CM[:], start=True, stop=True)
            GM = apool.tile([128, 128], F32, name="GM")
            nc.vector.tensor_mul(GM[:], gp[:], TRIU[:])
            ps_z = tc.tile_pool(name="ps_z", bufs=3, space="PSUM")
            psZ = ps_z.__enter__()
            for j in range(NA):
                zp = psZ.tile([128, TS], F32, name="zp", tag="zp", bufs=3)
                for k in range(NA):
                    nc.tensor.matmul(
                        zp[:], W_in[:, k, D + j * 128:D + (j + 1) * 128],
                        XT[:, k, HALO:HALO + TS], start=(k == 0), stop=(k == NA - 1))
                nc.vector.tensor_mul(ZSC[:, j, :], zp[:], SB[:, HALO:HALO + TS])
            ps_z.__exit__(None, None, None)
            SZ = apool.tile([128, NA, TS], F32, name="SZ")
            nc.scalar.activation(SZ[:], ZSC[:], AFT.Silu)
            Y1 = apool.tile([128, NA, 128], F32, name="Y1")
            for a in range(NA):
                y1p = psS.tile([128, 128], F32, name="y1p", tag="tw", bufs=3)
                nc.tensor.matmul(y1p[:], GM[:], WVT[:, a, :], start=True, stop=True)
                nc.vector.tensor_copy(Y1[:, a, :], y1p[:])

            # ---- AG readback + masked prefix combine ----
            GU = wpool.tile([128, NCORE, NA * DS + NA], F32)
            nc.sync.dma_start(GU[:], ag_out[:].rearrange("(j p) n -> p j n", p=128))
            S = apool.tile([128, NA * DS], F32, name="S")
            nc.vector.memset(S[:], 0.0)
            OFF = apool.tile([128, NA], F32, name="OFF")
            nc.vector.memset(OFF[:], 0.0)
            AJ = apool.tile([128, NA], F32, name="AJ")
            AJm = apool.tile([128, NA], F32, name="AJm")
            for j in range(NCORE - 1):
                nc.scalar.activation(AJ[:], GU[:, j, NA * DS:NA * DS + NA], AFT.Exp)
                # A'_j = A_j * m + (1 - m)
                nc.vector.tensor_scalar(AJm[:], AJ[:], CMASK[:, j:j + 1],
                                        CMASK[:, 8 + j:9 + j], OP.mult, OP.add)
                # S = S * bcast(A'_j) + U_j * m
                S3 = S[:].rearrange("p (a s) -> p a s", a=NA)
                nc.vector.tensor_mul(S3, S3, AJm[:].to_broadcast((128, NA, DS)))
                nc.vector.scalar_tensor_tensor(
                    S3, GU[:, j, 0:NA * DS].rearrange("p (a s) -> p a s", a=NA),
                    CMASK[:, j:j + 1], S3, OP.mult, OP.add)
                # off += la_sum_j * m
                nc.vector.scalar_tensor_tensor(
                    OFF[:], GU[:, j, NA * DS:NA * DS + NA], CMASK[:, j:j + 1],
                    OFF[:], OP.mult, OP.add)

            # poison mask from global cum = CUML + off
            MASK = apool.tile([128, NA, TS], mybir.dt.uint8, name="MASK")
            for a in range(NA):
                nc.vector.tensor_scalar(MASK[:, a, :], CUML[:, a, :],
                                        OFF[:, a:a + 1], THR, OP.add, OP.is_lt)

            # ---- y2 = Cm @ S^T, scale, transpose back ----
            YS = apool.tile([128, NA, TS], F32, name="YS")
            for a in range(NA):
                stp = psS.tile([DS, 128], F32, name="stp", tag="tw", bufs=3)
                nc.tensor.transpose(stp[:], S[:, a * DS:(a + 1) * DS], ident[:])
                ST = apool.tile([DS, 128], F32, name="ST", bufs=2)
                nc.vector.tensor_copy(ST[:], stp[:])
                yp = psS.tile([128, 128], F32, name="yp", tag="yp", bufs=2)
                nc.tensor.matmul(yp[:], CM[:], ST[:], start=True, stop=True)
                YT = apool.tile([128, 128], F32, name="YT", bufs=2)
                nc.vector.tensor_add(YT[:], yp[:], Y1[:, a, :])
                nc.vector.tensor_mul(YT[:], YT[:], PTT[:, a, :])
                yb = psS.tile([128, 128], F32, name="yb", tag="tw", bufs=3)
                nc.tensor.transpose(yb[:], YT[:], ident[:])
                nc.vector.tensor_copy(YS[:, a, :], yb[:])
            ps_s.__exit__(None, None, None)

            # ---- gating + poison + out_proj ----
            YD = apool.tile([128, NA, TS], F32, name="YD")
            for a in range(NA):
                nc.vector.scalar_tensor_tensor(YD[:, a, :], XS2[:, a, :],
                                               PPT[:, a, 3:4], YS[:, a, :],
                                               OP.mult, OP.add)
            YF = apool.tile([128, NA, TS], F32, name="YF")
            nc.vector.tensor_mul(YF[:], YD[:], SZ[:])
            nc.vector.copy_predicated(
                YF[:].rearrange("p a t -> p (a t)"),
                MASK[:].rearrange("p a t -> p (a t)"), nan_t[:])
            YFB = apool.tile([128, NA, TS], BF16, name="YFB")
            nc.vector.tensor_copy(YFB[:], YF[:])

            XRES = wpool.tile([128, NA, TS], F32)
            nc.sync.dma_start(XRES[:], xres.ap().rearrange("p (a n) -> p a n", a=NA))
            ps_o = tc.tile_pool(name="ps_o", bufs=8, space="PSUM")
            psO = ps_o.__enter__()
            OUT = apool.tile([128, NA, TS], F32, name="OUT")
            for j in range(NA):
                op_ = psO.tile([128, TS], F32, name="op", tag="op", bufs=8)
                for k in range(NA):
                    nc.tensor.matmul(op_[:], W_out[:, k, j * 128:(j + 1) * 128],
                                     YFB[:, k, :], start=(k == 0), stop=(k == NA - 1))
                nc.vector.tensor_add(OUT[:, j, :], op_[:], XRES[:, j, :])
            ps_o.__exit__(None, None, None)
            nc.sync.dma_start(out.ap().rearrange("p (a n) -> p a n", a=NA), OUT[:])

    nc.compile()
    _CACHE["nc"] = nc
    return nc


def kernel(x, norm_w, in_proj_w, conv_w, conv_b, dt_w, dt_b, B_w, C_w, out_w,
           log_A, D: np.ndarray = None, **kw):
    import ml_dtypes
    global LAST_RESULT
    bf = ml_dtypes.bfloat16
    Dv = D if D is not None else kw["D"]
    f32 = np.float32

    nc = _build()
    xT = np.ascontiguousarray(np.asarray(x, f32)[0].T)          # (d, T)
    xTp = np.concatenate([np.zeros((1024, HALO), f32), xT], axis=1)
    nw = np.asarray(norm_w, f32)
    W_in_f = np.asarray(in_proj_w, f32) * nw[None, :]
    w_inT = np.ascontiguousarray(W_in_f.T).astype(bf)           # (d, 2d)
    dt_wTf = np.ascontiguousarray(np.asarray(dt_w, f32).T).astype(bf)
    out_wTf = np.ascontiguousarray(np.asarray(out_w, f32).T).astype(bf)
    bc_wT = np.ascontiguousarray(
        np.concatenate([np.asarray(B_w, f32).T, np.asarray(C_w, f32).T],
                       axis=1)).astype(bf)
    A = (-np.exp(np.asarray(log_A, f32))).astype(f32)
    cw = np.asarray(conv_w, f32)[:, 0, :]
    ppt = np.stack([A, np.asarray(dt_b, f32), np.asarray(conv_b, f32),
                    np.asarray(Dv, f32), cw[:, 0], cw[:, 1], cw[:, 2],
                    cw[:, 3]], axis=1).astype(np.float32)        # (d, 8)
    ppt = np.ascontiguousarray(
        ppt.reshape(NA, 128, 8).transpose(1, 0, 2).reshape(128, NA * 8))
    triu_m = np.triu(np.ones((128, 128), np.float32))

    in_maps = []
    for k in range(NCORE):
        t0 = k * TS
        xs_slice = np.zeros((1024, 136), np.float32)
        xs_slice[:, 0:TS + HALO] = xTp[:, t0:t0 + TS + HALO]
        xs_slice = xs_slice.astype(bf)
        cm = np.zeros((128, 16), np.float32)
        cm[:, 0:NCORE - 1] = (np.arange(NCORE - 1) < k).astype(np.float32)[None, :]
        cm[:, 8:8 + NCORE - 1] = 1.0 - cm[:, 0:NCORE - 1]
        in_maps.append(dict(
            xTs=xs_slice,
            xres=np.ascontiguousarray(
                xT[:, t0:t0 + TS].reshape(NA, 128, TS).transpose(1, 0, 2)
                .reshape(128, NA * TS)),
            w_inT=w_inT, dt_wT=dt_wTf, bc_wT=bc_wT, out_wT=out_wTf,
            ppt=ppt, cmask=cm, triu=triu_m))

    res = bass_utils.run_bass_kernel_spmd(nc, in_maps,
                                          core_ids=list(range(NCORE)),
                                          trace=TRACE)
    LAST_RESULT = res
    cols = []
    for k in range(NCORE):
        o = res.results[k]["out"].reshape(128, NA, TS).transpose(1, 0, 2)
        cols.append(o.reshape(1024, TS))
    full = np.concatenate(cols, axis=1)
    return np.ascontiguousarray(full.T)[None].astype(np.float32)
